# revision 10
# baseline (speedup 1.0000x reference)
"""CMAttention Trainium2 Bass kernel (v2).

Reference computation (b=2, n=2048, dim=512, H=8 heads, dh=64, M=3 memory tokens):
    q = x @ wq;  k, v = split(x @ wkv);  per-head attention with 3 extra
    memory k/v tokens appended;  out = softmax(q k^T / 8) v;  y = out @ wo + bo.

Sharding: 16 (batch, head) pairs over 8 cores -> each core owns one batch and
two adjacent heads.  Per core everything is local; the out-projection is
row-sharded (per-head) and partial outputs are summed on the host.

v2 design (vs v1 at ~139us):
  * everything bf16 on the PE (x, weights, q, k, v, exp, oT) - fp8 was tested
    numerically and blows the 2e-2 gate, so PE streaming floor ~72K cycles.
  * ACT does ONLY exp (68 x [128,1024] tiles ~70us busy) - all copies ride on
    DVE/Pool; out-DMA triggers on sync/gpsimd queues.
  * early exp start: first-column-window DMAs on 4 queues, kT/q chunk 0
    projected immediately, first exp ~4us.
  * out-projection: oT is normalized in place (reciprocal row broadcast via
    gpsimd partition_broadcast), so one 128-contraction matmul per token tile
    (both heads at once) replaces v1's two half-matmuls + PSUM rescale.
  * slot scheduler: per j-tile slot emits [av-drain, one extra unit, scores];
    extra units = deferred kT/qT/vT chunks, v transposes, out-projection
    tiles, spread so PE never idles and ACT never starves.
PSUM: sp 2x[128,1024] (4 banks) + av 2x[65,512] (2) + mix pool (2).
"""

import sys

for _p in ("/opt/trn_rl_repo", "/root/.axon_site/_ro/trn_rl_repo"):
    if _p not in sys.path:
        sys.path.insert(0, _p)

import ml_dtypes
import numpy as np

import concourse.bacc as bacc
import concourse.mybir as mybir
import concourse.tile as tile
from concourse import bass_utils
from concourse.masks import make_identity

F32 = mybir.dt.float32
BF = mybir.dt.bfloat16
AF = mybir.ActivationFunctionType
ALU = mybir.AluOpType

H, DH, M = 8, 64, 3
DIM = 512
INNER = H * DH
NSEQ = 2048
B = 2
N_CORES = 8
SCALE = DH ** -0.5
SQRT_M = float(np.sqrt(M))

_CACHE = {}


def _emit(nc, tc, n):
    n_it = n // 128          # 16 token tiles
    n_iq = n // 512          # 4 query quarters
    n_jt = n // 128 + 1      # 17 j-tiles (16 regular + memory)
    VA = 65                  # v_aug cols per j-tile: 64 dims + ones column

    ap_xt = nc.dram_tensor("xt", [4, 128, n], BF, kind="ExternalInput").ap()
    ap_wq = nc.dram_tensor("wq_s", [128, DIM], BF, kind="ExternalInput").ap()
    ap_wk = nc.dram_tensor("wk_s", [128, DIM], BF, kind="ExternalInput").ap()
    ap_wv = nc.dram_tensor("wv_s", [128, DIM], BF, kind="ExternalInput").ap()
    ap_wo = nc.dram_tensor("wo_s", [128, DIM], BF, kind="ExternalInput").ap()
    ap_mkT = nc.dram_tensor("mkT_s", [128, M], BF, kind="ExternalInput").ap()
    ap_mv = nc.dram_tensor("mv_s", [M, 128], BF, kind="ExternalInput").ap()
    ap_out = nc.dram_tensor("out", [n_it, 128, DIM], F32, kind="ExternalOutput").ap()

    with (
        tc.tile_pool(name="persist", bufs=1) as per,
    ):
        xt = [per.tile([128, n], BF, tag=f"xt{c}", name=f"xt{c}") for c in range(4)]
        wq_sb = per.tile([128, DIM], BF, tag="wq", name="wq")
        wk_sb = per.tile([128, DIM], BF, tag="wk", name="wk")
        wv_sb = per.tile([128, DIM], BF, tag="wv", name="wv")
        wo_sb = per.tile([128, DIM], BF, tag="wo", name="wo")
        qT = per.tile([128, n], BF, tag="qT", name="qT")
        kT = per.tile([128, n + 128], BF, tag="kT", name="kT")
        vT = per.tile([128, n], BF, tag="vT", name="vT")
        v_aug = [per.tile([128, n_jt * VA], BF, tag=f"vaug{h}", name=f"vaug{h}") for h in range(2)]
        oT = per.tile([128, n], BF, tag="oT", name="oT")
        rec_sb = [per.tile([1, 512], F32, tag=f"rsb{h}", name=f"rsb{h}") for h in range(2)]
        rec_b = [per.tile([128, 512], F32, tag=f"rb{h}", name=f"rb{h}") for h in range(2)]
        ident = per.tile([128, 128], BF, tag="ident", name="ident")

        # ---- prologue DMAs: first 512-col window of each xt chunk on its own
        # queue, weights interleaved, then the rest of xt.
        cw = 512
        qs = [nc.sync, nc.scalar, nc.gpsimd, nc.sync]
        for c in range(4):
            qs[c].dma_start(out=xt[c][:, 0:cw], in_=ap_xt[c][:, 0:cw])
        nc.sync.dma_start(out=wk_sb, in_=ap_wk)
        nc.scalar.dma_start(out=wq_sb, in_=ap_wq)
        nc.gpsimd.dma_start(out=wo_sb, in_=ap_wo)
        for c in range(4):
            qs[c].dma_start(out=xt[c][:, cw:], in_=ap_xt[c][:, cw:])
        nc.scalar.dma_start(out=wv_sb, in_=ap_wv)
        nc.vector.memset(kT[:, n : n + 128], 0.0)
        nc.sync.dma_start(out=kT[:, n : n + M], in_=ap_mkT)
        make_identity(nc, ident[:])
        for h in range(2):
            nc.vector.memset(v_aug[h][:], 1.0)
            mb = (n_jt - 1) * VA
            nc.vector.memset(v_aug[h][:, mb : mb + VA], 0.0)
            nc.vector.memset(v_aug[h][0:M, mb + 64 : mb + VA], 1.0)
            nc.gpsimd.dma_start(
                out=v_aug[h][0:M, mb : mb + 64],
                in_=ap_mv[:, h * 64 : (h + 1) * 64],
            )

        with (
            tc.tile_pool(name="sp_ps", bufs=2, space="PSUM") as sp_pool,
            tc.tile_pool(name="av_ps", bufs=2, space="PSUM") as av_pool,
            tc.tile_pool(name="mix_ps", bufs=2, space="PSUM") as mix_ps,
            tc.tile_pool(name="et_sb", bufs=12) as et_pool,
            tc.tile_pool(name="stage", bufs=6) as stage,
        ):
            # ---------- unit helpers (each ~<=1 slot of PE work) ----------
            def proj_chunk(dst, w_sb, ic, copy_eng=None):
                ps = mix_ps.tile([128, 512], F32, tag="mix", name="pu")
                for c in range(4):
                    nc.tensor.matmul(
                        ps[:],
                        w_sb[:, c * 128 : (c + 1) * 128],
                        xt[c][:, ic * 512 : (ic + 1) * 512],
                        start=(c == 0),
                        stop=(c == 3),
                    )
                # gpsimd cannot touch PSUM; DVE handles all PSUM->SBUF moves
                nc.vector.tensor_copy(
                    out=dst[:, ic * 512 : (ic + 1) * 512], in_=ps[:]
                )

            def tr_group(g):
                # transpose token tiles 4g..4g+3 of vT into v_aug blocks
                for k in range(4):
                    tt = 4 * g + k
                    pt = mix_ps.tile([128, 128], BF, tag="mix", name="tr")
                    nc.tensor.transpose(
                        pt[:], vT[:, tt * 128 : (tt + 1) * 128], ident[:]
                    )
                    for h in range(2):
                        nc.vector.tensor_copy(
                            out=v_aug[h][:, tt * VA : tt * VA + 64],
                            in_=pt[:, h * 64 : (h + 1) * 64],
                        )

            def normalize(iq):
                assert av_cnt.get(iq) == n_jt, (
                    f"normalize({iq}) before its av epilogue was emitted"
                )
                for h in range(2):
                    sl = oT[h * 64 : (h + 1) * 64, iq * 512 : (iq + 1) * 512]
                    nc.vector.tensor_tensor(
                        out=sl,
                        in0=sl,
                        in1=rec_b[h][h * 64 : (h + 1) * 64, :],
                        op=ALU.mult,
                    )

            def outproj_tile(t, k):
                p = mix_ps.tile([128, 512], F32, tag="mix", name="op")
                nc.tensor.matmul(
                    p[:],
                    oT[:, t * 128 : (t + 1) * 128],
                    wo_sb[:],
                    start=True,
                    stop=True,
                )
                outb = stage.tile([128, 512], F32, tag="outb", name="outb")
                if k % 2 == 0:
                    nc.vector.tensor_copy(out=outb[:], in_=p[:])
                else:
                    nc.scalar.copy(out=outb[:], in_=p[:])
                deng = nc.sync if k % 2 == 0 else nc.gpsimd
                deng.dma_start(out=ap_out[t], in_=outb[:])

            # ---------- av bookkeeping ----------
            avs_of = {}
            av_cnt = {}
            pend = []           # (iq, jt, et) awaiting av
            tr_done = [False] * 5

            def av_ready(iq, jt):
                return jt == n_jt - 1 or tr_done[jt // 4]

            def emit_av():
                iq, jt, et = pend.pop(0)
                if iq not in avs_of:
                    avs_of[iq] = [
                        av_pool.tile([VA, 512], F32, tag="av", name=f"av{iq}_{h}")
                        for h in range(2)
                    ]
                    av_cnt[iq] = 0
                first = av_cnt[iq] == 0
                last = av_cnt[iq] == n_jt - 1
                for h in range(2):
                    nc.tensor.matmul(
                        avs_of[iq][h][:],
                        v_aug[h][:, jt * VA : (jt + 1) * VA],
                        et[:, h * 512 : (h + 1) * 512],
                        start=first,
                        stop=last,
                    )
                av_cnt[iq] += 1
                if last:
                    # epilogue: unnormalized oT copy (frees the banks) + the
                    # reciprocal row + its partition broadcast.
                    for h in range(2):
                        nc.vector.tensor_copy(
                            out=oT[h * 64 : (h + 1) * 64, iq * 512 : (iq + 1) * 512],
                            in_=avs_of[iq][h][0:64, :],
                        )
                        nc.vector.reciprocal(
                            out=rec_sb[h][:], in_=avs_of[iq][h][64:65, :]
                        )
                        nc.gpsimd.partition_broadcast(rec_b[h][:], rec_sb[h][0:1, :])

            # ---------- prologue compute: kT chunk 0 + qT chunk 0 ----------
            proj_chunk(kT, wk_sb, 0, nc.vector)
            proj_chunk(qT, wq_sb, 0, nc.gpsimd)

            # ---------- unit schedules ----------
            def mk_units(iq):
                if iq == 0:
                    u = [
                        lambda: proj_chunk(kT, wk_sb, 1, nc.vector),
                        lambda: proj_chunk(kT, wk_sb, 2, nc.gpsimd),
                        lambda: proj_chunk(vT, wv_sb, 0, nc.vector),
                        lambda: proj_chunk(kT, wk_sb, 3, nc.gpsimd),
                        lambda: (tr_group(0), tr_done.__setitem__(0, True)),
                        lambda: proj_chunk(vT, wv_sb, 1, nc.vector),
                        lambda: (tr_group(1), tr_done.__setitem__(1, True)),
                        lambda: proj_chunk(vT, wv_sb, 2, nc.gpsimd),
                        lambda: (tr_group(2), tr_done.__setitem__(2, True)),
                        lambda: proj_chunk(vT, wv_sb, 3, nc.vector),
                        lambda: (tr_group(3), tr_done.__setitem__(3, True)),
                        lambda: proj_chunk(qT, wq_sb, 1, nc.gpsimd),
                    ]
                    # one unit per slot from slot 0
                    return {s: u[s] for s in range(len(u))}
                # iq >= 1: normalize prev quarter, then its 4 outproj tiles;
                # qT chunk iq+1 late in the quarter.
                prev = iq - 1
                sched = {}
                sched[4] = lambda: normalize(prev)
                for k in range(4):
                    t = prev * 4 + k
                    sched[6 + 2 * k] = (lambda t=t, k=k: outproj_tile(t, k))
                if iq + 1 < n_iq:
                    sched[14] = lambda iq=iq: proj_chunk(qT, wq_sb, iq + 1, nc.vector)
                return sched

            # ---------- main loop ----------
            for iq in range(n_iq):
                units = mk_units(iq)
                for jt in range(n_jt):
                    # 1. av drain (oldest first)
                    budget = 1 if len(pend) < 7 else 2
                    while budget > 0 and pend and av_ready(*pend[0][:2]):
                        emit_av()
                        budget -= 1
                    # 2. scheduled extra unit
                    if jt in units:
                        units[jt]()
                    # 3. scores + exp for this j-tile
                    sp = sp_pool.tile([128, 1024], F32, tag="sp", name="sp")
                    for h in range(2):
                        hp = h * 64
                        nc.tensor.matmul(
                            sp[:, h * 512 : (h + 1) * 512],
                            kT[hp : hp + 64, jt * 128 : (jt + 1) * 128],
                            qT[hp : hp + 64, iq * 512 : (iq + 1) * 512],
                            start=True,
                            stop=True,
                        )
                    et = et_pool.tile([128, 1024], BF, tag="exp", name="et")
                    nc.scalar.activation(out=et[:], in_=sp[:], func=AF.Exp)
                    pend.append((iq, jt, et))

            # ---------- tail: drain + final quarter epilogue/outproj ----------
            while pend:
                emit_av()
            normalize(n_iq - 1)
            for k in range(4):
                outproj_tile((n_iq - 1) * 4 + k, k)


def _build(n=NSEQ):
    if n in _CACHE:
        return _CACHE[n]
    nc = bacc.Bacc("TRN2", debug=False, num_devices=N_CORES)
    with tile.TileContext(nc) as tc:
        _emit(nc, tc, n)
    nc.compile()
    _CACHE[n] = nc
    return nc


def _prep_in_maps(x, wq, wkv, wo, m_k, m_v, n):
    x = np.asarray(x, np.float32)
    wq = np.asarray(wq, np.float32)
    wkv = np.asarray(wkv, np.float32)
    wo = np.asarray(wo, np.float32)
    m_k = np.asarray(m_k, np.float32)
    m_v = np.asarray(m_v, np.float32)

    wk = wkv[:, :INNER]
    wv = wkv[:, INNER:]
    # memory tokens: flat reshape (M, INNER) -> (H, M, DH), exactly as reference
    mk_heads = m_k.reshape(M * INNER).reshape(H, M, DH)  # * SQRT_DH * SCALE == 1.0
    mv_heads = m_v.reshape(M * INNER).reshape(H, M, DH) * SQRT_M

    def wchunks(w):
        # [512, 128] -> [128, 4*128] with chunk c at cols [c*128,(c+1)*128)
        return np.ascontiguousarray(
            w.reshape(4, 128, 128).transpose(1, 0, 2).reshape(128, DIM)
        ).astype(ml_dtypes.bfloat16)

    in_maps = []
    for cid in range(N_CORES):
        b = cid // 4
        h0 = 2 * (cid % 4)
        sl = slice(h0 * DH, (h0 + 2) * DH)
        in_maps.append(
            {
                "xt": np.ascontiguousarray(x[b].T)
                .reshape(4, 128, n)
                .astype(ml_dtypes.bfloat16),
                "wq_s": wchunks(wq[:, sl]),
                "wk_s": wchunks(wk[:, sl] * SCALE),
                "wv_s": wchunks(wv[:, sl]),
                "wo_s": np.ascontiguousarray(wo[sl, :]).astype(ml_dtypes.bfloat16),
                "mkT_s": np.ascontiguousarray(
                    np.concatenate([mk_heads[h0].T, mk_heads[h0 + 1].T], axis=0)
                ).astype(ml_dtypes.bfloat16),
                "mv_s": np.ascontiguousarray(
                    np.concatenate([mv_heads[h0], mv_heads[h0 + 1]], axis=1)
                ).astype(ml_dtypes.bfloat16),
            }
        )
    return in_maps


def _gather(results, bo, n):
    bo = np.asarray(bo, np.float32)
    out = np.zeros((B, n, DIM), np.float32)
    for cid in range(N_CORES):
        out[cid // 4] += results[cid]["out"].reshape(n, DIM)
    out += bo
    return out


def run(x, wq, wkv, wo, bo, m_k, m_v, trace=False, n=NSEQ):
    nc = _build(n)
    in_maps = _prep_in_maps(x, wq, wkv, wo, m_k, m_v, n)
    res = bass_utils.run_bass_kernel_spmd(
        nc, in_maps, core_ids=list(range(N_CORES)), trace=trace
    )
    return _gather(res.results, bo, n), res


def kernel(x, wq, wkv, wo, bo, m_k, m_v):
    out, _ = run(x, wq, wkv, wo, bo, m_k, m_v)
    return out
